# revision 7
# baseline (speedup 1.0000x reference)
"""Adversarial loss kernel for Trainium2 (8 NeuronCores, data-parallel).

Computes, for pred [4096, 32000] f32 and target [4096] int:
    out[b] = -(sum_c log(sigmoid(pred[b,c])) - log(sigmoid(pred[b,target[b]]))) / C
           = (sum_c softplus(-pred[b,c]) - softplus(-pred[b,target[b]])) / C

Sharding: pure data parallel over the batch dim — 512 rows per core.
Per core the kernel streams its [512, 32000] shard through SBUF in
[128, CT] tiles, fusing softplus(-x) + row-sum into single ScalarE
ACTIVATE instructions via accum_out.  The target entry of each row is
fetched with an indirect (gather) DMA and its softplus is subtracted.
"""

import sys

sys.path.insert(0, "/opt/trn_rl_repo")

import numpy as np

from concourse import bass, bacc, mybir
import concourse.tile as tile
from concourse.bass_utils import run_bass_kernel_spmd

B, C = 4096, 32000
NCORES = 8
R = B // NCORES  # rows per core
P = 128  # SBUF partitions
NRB = R // P  # row blocks per core
CT = 2000  # column-tile width
NCT = C // CT  # column tiles per row block

F32 = mybir.dt.float32
I32 = mybir.dt.int32
EXP = mybir.ActivationFunctionType.Exp
LN = mybir.ActivationFunctionType.Ln


def build_nc():
    nc = bacc.Bacc(None, target_bir_lowering=False)
    pred = nc.declare_dram_parameter("pred", [R, C], F32, isOutput=False)
    gidx = nc.declare_dram_parameter("gidx", [R], I32, isOutput=False)
    out = nc.declare_dram_parameter("out", [R], F32, isOutput=True)

    # Flat [R*C, 1] view of pred for the element gather.
    pred_flat = pred[:, :].rearrange("a b -> (a b)")[:, None]

    with tile.TileContext(nc) as tc:
        with (
            tc.tile_pool(name="pin", bufs=6) as pin,
            tc.tile_pool(name="psc", bufs=3) as psc,
            tc.tile_pool(name="psm", bufs=3) as psm,
        ):
            for rb in range(NRB):
                rows = slice(rb * P, (rb + 1) * P)
                partials = psm.tile([P, NCT], F32)
                for ct in range(NCT):
                    t = pin.tile([P, CT], F32, tag="in")
                    nc.sync.dma_start(
                        out=t[:], in_=pred[rows, ct * CT : (ct + 1) * CT]
                    )
                    # e = exp(-x); softplus(-x) = ln(1 + e).  Exp and Ln live
                    # in the same ACT table set (natural_log_exp_and_others).
                    e = psc.tile([P, CT], F32, tag="e")
                    nc.scalar.activation(out=e[:], in_=t[:], func=EXP, scale=-1.0)
                    s = psc.tile([P, CT], F32, tag="scratch")
                    nc.scalar.activation(
                        out=s[:],
                        in_=e[:],
                        func=LN,
                        bias=1.0,
                        accum_out=partials[:, ct : ct + 1],
                    )
                idx_t = psm.tile([P, 1], I32)
                nc.sync.dma_start(out=idx_t[:], in_=gidx[rows, None])
                tv = psm.tile([P, 1], F32)
                nc.gpsimd.indirect_dma_start(
                    out=tv[:],
                    out_offset=None,
                    in_=pred_flat,
                    in_offset=bass.IndirectOffsetOnAxis(ap=idx_t[:, :1], axis=0),
                )
                e_t = psm.tile([P, 1], F32)
                nc.scalar.activation(out=e_t[:], in_=tv[:], func=EXP, scale=-1.0)
                corr = psm.tile([P, 1], F32)
                nc.scalar.activation(out=corr[:], in_=e_t[:], func=LN, bias=1.0)
                rsum = psm.tile([P, 1], F32)
                nc.vector.reduce_sum(
                    out=rsum[:], in_=partials[:], axis=mybir.AxisListType.X
                )
                d = psm.tile([P, 1], F32)
                nc.vector.tensor_sub(d[:], rsum[:], corr[:])
                o = psm.tile([P, 1], F32)
                nc.scalar.mul(o[:], d[:], 1.0 / C)
                nc.sync.dma_start(out=out[rows, None], in_=o[:])
    nc.finalize()
    return nc


_NC = None


def _get_nc():
    global _NC
    if _NC is None:
        _NC = build_nc()
    return _NC


def _make_in_maps(pred, target):
    pred = np.ascontiguousarray(np.asarray(pred, dtype=np.float32))
    tgt = np.asarray(target).astype(np.int64)
    in_maps = []
    for c in range(NCORES):
        rs = c * R
        loc_t = tgt[rs : rs + R]
        g = (np.arange(R, dtype=np.int64) * C + loc_t).astype(np.int32)
        in_maps.append({"pred": pred[rs : rs + R], "gidx": g})
    return in_maps


def kernel(pred, target, _trace=False):
    nc = _get_nc()
    in_maps = _make_in_maps(pred, target)
    res = run_bass_kernel_spmd(
        nc, in_maps, core_ids=list(range(NCORES)), trace=_trace
    )
    out = np.concatenate([res.results[i]["out"] for i in range(NCORES)])
    if _trace:
        kernel.last_results = res
    return out.astype(np.float32)


# revision 9
# speedup vs baseline: 1.9500x; 1.9500x over previous
"""Adversarial loss kernel for Trainium2 (8 NeuronCores, data-parallel).

For pred [4096, 32000] f32 and target [4096] int:
    out[b] = -(sum_c log(sigmoid(pred[b,c])) - log(sigmoid(pred[b,target[b]]))) / C

Sharding: pure data parallel over the batch dim — 512 rows per core.

Per-core pipeline (memory-bound problem; ~65.5 MB of pred per core):
  1. DMA [128, CT] tiles of pred into SBUF.
  2. ScalarE ACT computes sigmoid(x) per tile — a single activation
     function for the whole bulk pass, so the ACT table is loaded once.
  3. VectorE reduces groups of 8 sigmoids with a product (ln prod sigma =
     sum ln sigma; groups of 8 keep the product in f32 range).
  4. The target entry of each row is fetched by indirect-gather DMA;
     1/sigmoid(x_t) is appended as one extra product column — its ln
     contributes exactly -ln sigmoid(x_t).
  5. One LN+accumulate activation per row block over the product columns
     yields sum_c ln sigmoid - ln sigmoid_t; scale by -1/C.
"""

import sys

sys.path.insert(0, "/opt/trn_rl_repo")

import numpy as np

from concourse import bass, bacc, mybir
import concourse.tile as tile
from concourse.bass_utils import run_bass_kernel_spmd

B, C = 4096, 32000
NCORES = 8
R = B // NCORES  # rows per core
P = 128  # SBUF partitions
NRB = R // P  # row blocks per core
CT = 2000  # column-tile width
NCT = C // CT  # column tiles per row block
GRP = 8  # sigmoid-product group size
NG = CT // GRP  # product columns per tile
NGR = NCT * NG  # product columns per row block

F32 = mybir.dt.float32
I32 = mybir.dt.int32
SIG = mybir.ActivationFunctionType.Sigmoid
LN = mybir.ActivationFunctionType.Ln


def build_nc():
    nc = bacc.Bacc(None, target_bir_lowering=False)
    pred = nc.declare_dram_parameter("pred", [R, C], F32, isOutput=False)
    gidx = nc.declare_dram_parameter("gidx", [R], I32, isOutput=False)
    out = nc.declare_dram_parameter("out", [R], F32, isOutput=True)

    # Flat [R*C, 1] view of pred for the target-element gather.
    pred_flat = pred[:, :].rearrange("a b -> (a b)")[:, None]

    with tile.TileContext(nc) as tc:
        with (
            tc.tile_pool(name="pin", bufs=6) as pin,
            tc.tile_pool(name="psg", bufs=4) as psg,
            tc.tile_pool(name="pg", bufs=1) as pg,
            tc.tile_pool(name="psm", bufs=2) as psm,
        ):
            # Gather pred[r, target[r]] for all rows: one [P, 1] indirect
            # DMA per row block into a shared [P, NRB] tile.
            tv = psm.tile([P, NRB], F32, tag="tv")
            for rb in range(NRB):
                idx_t = psm.tile([P, 1], I32, tag=f"idx{rb}")
                nc.sync.dma_start(
                    out=idx_t[:], in_=gidx[rb * P : (rb + 1) * P, None]
                )
                nc.gpsimd.indirect_dma_start(
                    out=tv[:, rb : rb + 1],
                    out_offset=None,
                    in_=pred_flat,
                    in_offset=bass.IndirectOffsetOnAxis(ap=idx_t[:, :1], axis=0),
                )
            sgt = psm.tile([P, NRB], F32, tag="sgt")
            nc.scalar.activation(out=sgt[:], in_=tv[:], func=SIG)

            # One product tile per row block: NGR group products plus one
            # correction column holding 1/sigmoid(x_t).
            gt = []
            for rb in range(NRB):
                g_rb = pg.tile([P, NGR + 1], F32, tag=f"g{rb}")
                gt.append(g_rb)
            for rb in range(NRB):
                nc.vector.reciprocal(
                    out=gt[rb][:, NGR : NGR + 1], in_=sgt[:, rb : rb + 1]
                )

            for rb in range(NRB):
                rows = slice(rb * P, (rb + 1) * P)
                for ct in range(NCT):
                    t = pin.tile([P, CT], F32, tag="in")
                    nc.sync.dma_start(
                        out=t[:], in_=pred[rows, ct * CT : (ct + 1) * CT]
                    )
                    s = psg.tile([P, CT], F32, tag="sig")
                    nc.scalar.activation(out=s[:], in_=t[:], func=SIG)
                    nc.vector.tensor_reduce(
                        out=gt[rb][:, ct * NG : (ct + 1) * NG],
                        in_=s[:].rearrange("p (g k) -> p g k", k=GRP),
                        op=mybir.AluOpType.mult,
                        axis=mybir.AxisListType.X,
                    )

            # ln of all product columns, accumulated per row -> the loss.
            for rb in range(NRB):
                rows = slice(rb * P, (rb + 1) * P)
                lnout = psg.tile([P, NGR + 1], F32, tag="lnout")
                acc = psm.tile([P, 1], F32, tag="acc")
                nc.scalar.activation(
                    out=lnout[:], in_=gt[rb][:], func=LN, accum_out=acc[:]
                )
                o = psm.tile([P, 1], F32, tag="o")
                nc.scalar.mul(o[:], acc[:], -1.0 / C)
                nc.sync.dma_start(out=out[rows, None], in_=o[:])
    nc.finalize()
    return nc


_NC = None


def _get_nc():
    global _NC
    if _NC is None:
        _NC = build_nc()
    return _NC


def _make_in_maps(pred, target):
    pred = np.ascontiguousarray(np.asarray(pred, dtype=np.float32))
    tgt = np.asarray(target).astype(np.int64)
    in_maps = []
    for c in range(NCORES):
        rs = c * R
        loc_t = tgt[rs : rs + R]
        g = (np.arange(R, dtype=np.int64) * C + loc_t).astype(np.int32)
        in_maps.append({"pred": pred[rs : rs + R], "gidx": g})
    return in_maps


def kernel(pred, target, _trace=False):
    nc = _get_nc()
    in_maps = _make_in_maps(pred, target)
    res = run_bass_kernel_spmd(
        nc, in_maps, core_ids=list(range(NCORES)), trace=_trace
    )
    out = np.concatenate([res.results[i]["out"] for i in range(NCORES)])
    if _trace:
        kernel.last_results = res
    return out.astype(np.float32)
